# revision 28
# baseline (speedup 1.0000x reference)
"""Trainium2 Bass kernel for nn_CustomParameterTransform (scatter_memory).

Reference semantics: coord_v [256, 30] holds 10 (x, y, mass) triplets per
sample. Each triplet maps to integer grid indices (x_i, y_i, m_i); a one-hot
volume z [B, 16, 128, 128] is scattered (z[b, m, y, x] = 1) and the output is
concat(1-z, z) over the channel axis -> [256, 32, 128, 128] f32 (512 MB).

Strategy (8 NeuronCores, batch-sharded, no cross-core comm):
  - The output is almost entirely constant: the first 16 channels are 1.0
    except at scatter points, the last 16 are 0.0 except at scatter points.
  - Per core (32 samples, 64 MB slab): fill the slab from constant SBUF
    tiles with large DMAs (write-only HBM traffic; ~425 GB/s sustained =
    16 SDMA engines x ~26.6 GB/s, the SBUF-AXI port limit), then fix up
    the 640 scatter points with indirect (scatter) DMAs on gpsimd/SWDGE.
  - Indices are computed on the host with the exact same jax ops as the
    reference (bit-identical floor/log10 behavior) and passed per-core as
    a [128, 8] int32 tensor of flat element offsets.

Trace-driven structure (each piece measured):
  - Both HWDGE queues (sync + scalar) carry 32 MB each and transition
    descriptor sizes IN LOCKSTEP (512 KB mini fills with 4 KB descs ->
    2 MB combo fills with 16 KB descs -> 4 MB mega fills with 32 KB
    descs -> 2 MB combo-sourced final fills). Windows where the two
    queues run different descriptor sizes measurably degrade all
    engines (~2x packet times); keeping them aligned sustains the
    ~425 GB/s ceiling.
  - First memsets on the vector engine (gpsimd wakes a little later) ->
    first fill at ~7.7 us instead of 8.4.
  - Scatter columns are aligned to the fill order: col j covers samples
    4j..4j+3, so cols 0-6 gate on fills that complete early/mid-stream
    and fully hide. Only col 7 (samples 28-31, the final fills) runs
    after the stream: one small scatter, ~2.6 us.
  - Light drain/barrier epilogue (see _light_drain_and_barrier). The
    remaining ~6 us exit cost (a NEFF-level sweep zeroing all event
    semaphores plus two exit barriers) is emitted by the NEFF toolchain
    downstream of bass and is not reachable from kernel code.
"""

import numpy as np

B = 256
NSRC = 10
NMC = 16
L = 128
NCORES = 8
BL = B // NCORES          # 32 samples per core
PLANE = L * L             # 16384
HALF = NMC * PLANE        # 262144 elements per half-slab
SLAB = 2 * HALF           # 524288 elements per sample
OUT_ELEMS = BL * SLAB     # 16777216 per core (64 MB)

N_SCATTER_COLS = 8        # col j covers samples 4j..4j+3

_CACHE = {}


def _build_nc():
    import concourse.bass as bass
    import concourse.tile as tile
    from concourse import bacc, mybir
    from concourse.tile_rust import add_dep_helper

    import types as _types
    from concourse.vector_clock import ScopedClock

    nc = bacc.Bacc("TRN2", target_bir_lowering=False, debug=False,
                   num_devices=NCORES)

    def _light_drain_and_barrier(self, tick_clock, wait_clock):
        """Replaces TileContext._drain_and_barrier for this kernel. The
        stock epilogue is drain + two all-engine EVSEM butterfly barriers
        around the sem clear. Requirements at kernel end are: (1) all DMA
        completions observed, (2) sems cleared for NEFF re-execution,
        (3) the clear happens after every engine's last sem use. (1) is
        the sync drain's global-clock waits; (3) is a counting-sem join
        (sync arrives only after the drain, so join>=4 implies all DMA
        done); (2) is the ranged clear. The second barrier is
        unnecessary: a re-execution cannot start until every engine --
        including the clearing gpsimd -- has ended."""
        nc_ = self.nc
        drain_inst = nc_.sync.drain()
        wait_clock.add_sem_waits(
            drain_inst.ins, ScopedClock({None: tick_clock.global_clock}))
        join = nc_.alloc_semaphore("tail_join")
        for eng in nc_.engines.values():
            if eng is not nc_.gpsimd:
                eng.sem_inc(join, 1)
        n_other = len(nc_.engines) - 1
        nc_.gpsimd.wait_ge(join, n_other)
        popped = nc_._tile_sem_poison_stack.pop()
        assert popped is self._sem_poison
        sems = list(self.sems.allocated().values())
        nc_.clear_and_free_semaphores(sems + [join])

    offs = nc.dram_tensor("offs", [128, N_SCATTER_COLS], mybir.dt.int32,
                          kind="ExternalInput").ap()
    out = nc.dram_tensor("out", [OUT_ELEMS], mybir.dt.float32,
                         kind="ExternalOutput").ap()

    with tile.TileContext(nc) as tc:
        tc._drain_and_barrier = _types.MethodType(_light_drain_and_barrier, tc)
        with tc.tile_pool(name="src", bufs=1) as src_pool, \
             tc.tile_pool(name="small", bufs=1) as small_pool:
            # Constant source tiles. Memset cost scales with the free-dim
            # cols (128 lanes run in parallel), so big tiles are split
            # column-wise between vector and gpsimd; the two minis are
            # split ACROSS gpsimd and vector (one each, their first
            # instruction in the block) so both queues' first fills
            # unblock as early as possible (~7.3/7.6 us).
            ones_mini = src_pool.tile([128, 1024], mybir.dt.float32)
            zeros_mini = src_pool.tile([128, 1024], mybir.dt.float32)
            nc.gpsimd.memset(ones_mini[:, :], 1.0)
            nc.vector.memset(zeros_mini[:, :], 0.0)
            # Scatter offsets load early on the gpsimd (SWDGE) queue --
            # done by ~10 us, well before the first scatter column gates
            # (~27 us). Its 128 tiny packets land in the source-limited
            # ramp where they cost nothing.
            offs_t = small_pool.tile([128, N_SCATTER_COLS], mybir.dt.int32)
            nc.gpsimd.dma_start(offs_t[:, :], offs[:, :])
            # combo: one full slab ([128, 4096]; DMA iterates partition-
            # major, so partitions 0-63 are the ones half, 64-127 zeros).
            # Feeds samples 2-5 early and samples 30-31 at the very end.
            combo_t = src_pool.tile([128, 4096], mybir.dt.float32)
            nc.vector.memset(combo_t[0:64, 0:2048], 1.0)
            nc.vector.memset(combo_t[64:128, 0:2048], 0.0)
            nc.gpsimd.memset(combo_t[0:64, 2048:4096], 1.0)
            nc.gpsimd.memset(combo_t[64:128, 2048:4096], 0.0)
            # mega: two slabs ([128, 8192]; slab = 64 partitions, ones iff
            # p%64 < 32). Feeds samples 6-29 as 4 MB pair fills.
            mega_t = src_pool.tile([128, 8192], mybir.dt.float32)
            for lo, hi, v in ((0, 32, 1.0), (32, 64, 0.0),
                              (64, 96, 1.0), (96, 128, 0.0)):
                nc.vector.memset(mega_t[lo:hi, 0:4096], v)
                nc.gpsimd.memset(mega_t[lo:hi, 4096:8192], v)

            # Scatter offsets: [128, 8] int32 flat element indices.
            # Columns are ALIGNED TO FILL ORDER: col j covers samples
            # 4j..4j+3 (rows 0-39 ones-half writes of 0.0, rows 40-79
            # z-half writes of 1.0), so each column's fill deps complete
            # early relative to the stream end -- except col 7, whose
            # samples are filled last by design (one small tail scatter).
            # (A column spanning 13 samples would gate on the latest of 7
            # fills -> a serialized gpsimd dispatch pileup at the tail.)
            # vals memsets ride on vector after its big memsets. Engine
            # ops must start at a partition multiple of 32, hence the
            # overwrite at rows 32:40.
            vals_t = small_pool.tile([128, N_SCATTER_COLS], mybir.dt.float32)
            nc.vector.memset(vals_t[0:32, :], 0.0)
            nc.vector.memset(vals_t[32:64, :], 1.0)
            nc.vector.memset(vals_t[32:40, :], 0.0)
            nc.vector.memset(vals_t[64:96, :], 1.0)

            MINI = 131072  # elements per mini fill (512 KB)
            ones_fills = {}   # sample -> list of fills covering its ones half
            zeros_fills = {}  # sample -> list of fills covering its zeros half

            # Samples 0-1 from the minis (ready first; 4 KB descriptors).
            # Each queue's FIRST fill (the ones-half lead fill; ones_mini
            # is memset ~0.3 us before zeros_mini so Tile dispatches these
            # first) is split into four 32-descriptor fills: HWDGE
            # descriptor generation for a 128-descriptor fill costs
            # ~0.7 us before the first byte moves, a 32-descriptor fill
            # ~0.2 us. Same 4 KB descriptor size, so the queues' size
            # lockstep is untouched.
            for s in (0, 1):
                e_ones = nc.sync if s == 0 else nc.scalar
                e_zeros = nc.scalar if s == 0 else nc.sync
                base = s * SLAB
                ones_fills[s] = [
                    e_ones.dma_start(
                        out[base + q * 32768:base + (q + 1) * 32768],
                        ones_mini[32 * q:32 * q + 32, :])
                    for q in range(4)]
                ones_fills[s].append(
                    e_ones.dma_start(out[base + MINI:base + 2 * MINI],
                                     ones_mini[:, :]))
                zeros_fills[s] = [
                    e_zeros.dma_start(
                        out[base + HALF + k * MINI:
                            base + HALF + (k + 1) * MINI],
                        zeros_mini[:, :])
                    for k in range(2)]
            # Samples 2-3 from combo (one 2 MB fill per queue, 16 KB
            # descriptors, simultaneously on both queues).
            for s in (2, 3):
                eng = nc.sync if s == 2 else nc.scalar
                f = eng.dma_start(out[s * SLAB:(s + 1) * SLAB], combo_t[:, :])
                ones_fills[s] = [f]
                zeros_fills[s] = [f]
            # Samples 4-31 from mega (4 MB pair fills, 32 KB descriptors),
            # ascending so scatter-column gating times are monotonic in
            # column index -- with the shared-AP WAW chain serializing
            # scatters in emission order, any ordering that makes a LOW
            # column gate late piles several chained scatters into the
            # tail (measured +2 us per extra link). The LAST fill on each
            # queue is one of col 7's pairs ((28,29) on sync, (30,31) on
            # scalar), so col 6's pairs end a full 4 MB before their
            # queue ends and every column except col 7 hides completely.
            # Both queues run 2MB@4K | 2MB@16K | 28MB@32K in lockstep.
            for s in range(4, 32, 2):
                eng = nc.sync if (s // 2) % 2 == 0 else nc.scalar
                f = eng.dma_start(out[s * SLAB:(s + 2) * SLAB], mega_t[:, :])
                for ss in (s, s + 1):
                    ones_fills[ss] = [f]
                    zeros_fills[ss] = [f]

            # Which sample-fills each scatter column touches.
            def deps(lo, hi):
                seen = {}
                for s in range(lo, hi):
                    for f in ones_fills[s] + zeros_fills[s]:
                        seen[id(f)] = f
                return list(seen.values())
            col_specs = [(slice(0, 80), deps(4 * j, 4 * j + 4))
                         for j in range(8)]

            # Narrow declared out AP ([1, 1] at offset 0, required by the
            # indirect API): the real write addresses come from the offset
            # tensor; a full-tensor AP would make Tile serialize every
            # scatter behind every fill (WAW), and the explicit col_deps
            # edges below provide the true ordering. (The shared AP does
            # WAW-chain the scatters behind each other, which is fine:
            # each chain link completes long before the next column's
            # fill deps, except the last -- and there is only one
            # tail-gated column.)
            out2d = out[0:1].unsqueeze(1)
            for j, (rows, fl_deps) in enumerate(col_specs):
                sc = nc.gpsimd.indirect_dma_start(
                    out=out2d,
                    out_offset=bass.IndirectOffsetOnAxis(
                        ap=offs_t[rows, j:j + 1], axis=0),
                    in_=vals_t[rows, j:j + 1],
                    in_offset=None,
                )
                for fl in fl_deps:
                    add_dep_helper(sc.ins, fl.ins,
                                   reason="scatter after its sample fills")

    nc.compile()
    return nc


def _compute_indices(coord_v, lows, highs, nmc, L_):
    """Replicates reference.py lines exactly (same jax ops on the default
    device) so the floor/log10 bin boundaries match bit-for-bit."""
    import jax.numpy as jnp

    cv = jnp.asarray(np.asarray(coord_v, dtype=np.float32))
    n = cv.shape[1] // 3
    v10 = cv.at[:, 2::3].set(jnp.log10(cv[:, 2::3]))
    lo = jnp.tile(jnp.asarray(np.asarray(lows, dtype=np.float32)), n)
    hi = jnp.tile(jnp.asarray(np.asarray(highs, dtype=np.float32)), n)
    coord_grid = (v10 - lo) / (hi - lo)
    tr = coord_grid.reshape(-1, 3)
    x_i = jnp.floor(tr[:, 0] * L_).astype(jnp.int32)
    y_i = jnp.floor(tr[:, 1] * L_).astype(jnp.int32)
    m_i = jnp.floor(tr[:, 2] * nmc).astype(jnp.int32)
    return (np.asarray(x_i), np.asarray(y_i), np.asarray(m_i))


def _prepare_in_maps(coord_v, lows, highs, nmc, L):
    nmc = int(nmc)
    L_ = int(L)
    x_i, y_i, m_i = _compute_indices(coord_v, lows, highs, nmc, L_)
    n_batch = coord_v.shape[0]
    n = coord_v.shape[1] // 3
    b_i = np.repeat(np.arange(n_batch, dtype=np.int64), n)

    # Flat element offsets (per core, local slab coordinates).
    flat_ones = ((b_i % BL) * SLAB + m_i.astype(np.int64) * PLANE
                 + y_i.astype(np.int64) * L_ + x_i.astype(np.int64))
    flat_z = flat_ones + HALF

    in_maps = []
    pts_per_core = BL * n  # 320
    for c in range(NCORES):
        sel = slice(c * pts_per_core, (c + 1) * pts_per_core)
        po = flat_ones[sel]
        pz = flat_z[sel]
        offs_np = np.zeros((128, N_SCATTER_COLS), dtype=np.int32)
        for j in range(8):   # col j: samples 4j..4j+3 (40 points)
            offs_np[0:40, j] = po[40 * j:40 * j + 40]
            offs_np[40:80, j] = pz[40 * j:40 * j + 40]
        in_maps.append({"offs": offs_np})
    return in_maps


def _run(in_maps, **kwargs):
    if "nc" not in _CACHE:
        _CACHE["nc"] = _build_nc()
    nc = _CACHE["nc"]
    from concourse.bass_utils import run_bass_kernel_spmd
    return run_bass_kernel_spmd(nc, in_maps, core_ids=list(range(NCORES)),
                                **kwargs)


def kernel(coord_v, lows, highs, nmc, L):
    nmc = int(nmc)
    L_ = int(L)
    assert nmc == NMC and L_ == globals()["L"], (nmc, L_)

    in_maps = _prepare_in_maps(coord_v, lows, highs, nmc, L_)
    res = _run(in_maps)
    parts = [res.results[c]["out"].reshape(BL, 2 * NMC, L_, L_)
             for c in range(NCORES)]
    return np.concatenate(parts, axis=0)
